# revision 23
# baseline (speedup 1.0000x reference)
"""Trainium2 Bass kernel for the 16-level ternary (Haar-style) wavelet
transform of f (len 3^16) with row-orthonormalized 3x3 Phi matrices.

Strategy:
  - Host: QR-orthonormalize the 3x3 Phi blocks; digit-reverse each
    3^7-element unit (ternary digit reversal) and lay each SBUF
    partition row out plane-major, so every level's three dot-product
    operands are contiguous thirds of the buffer.  That unlocks the
    DVE's packed bf16 modes (tensor_scalar ~3x, tensor_tensor ~1.7x)
    which strided access forfeits.
  - All signal data is bf16 (I/O DMA bytes halve); the 3x3 coefficients
    stay fp32 per-partition scalars.
  - Main SPMD kernel (8 cores): units of 3^7 = 2187 elems recurse
    levels 0..6 inside one partition; details stream straight out
    (levels 1+ via SBUF staging so the out-DMA count stays small).
  - Tail: level-7 signal f7 (19683 elems) is all-gathered in-NEFF and
    levels 7..15 run redundantly on every core (unit averages are
    order-independent, so digit reversal does not affect f7).
"""

import sys

for _p in ("/opt/trn_rl_repo",):
    if _p not in sys.path:
        sys.path.append(_p)

import numpy as np
import ml_dtypes

import concourse.bass as bass
import concourse.mybir as mybir
import concourse.tile as tile
from concourse.bass_utils import run_bass_kernel_spmd

F32 = mybir.dt.float32
BF16 = mybir.dt.bfloat16
NPBF = ml_dtypes.bfloat16
MULT = mybir.AluOpType.mult
ADD = mybir.AluOpType.add

NL = 16                   # total levels
LK = 7                    # levels computed by the main kernel (0..6)
UNIT = 3 ** LK            # 2187 input elems per unit
NUNITS = 3 ** (NL - LK)   # 19683 units overall
NCORES = 8
UPP = 4                   # units per partition per tile
T = 5                     # tiles per core
PAD_UNITS = T * 128 * UPP  # 2560 padded units per core

# contiguous unit ranges per core (2461 x7 + 2456)
_base = [0]
for _k in range(NCORES):
    _base.append(_base[-1] + (2461 if _k < 7 else NUNITS - 7 * 2461))
CORE_U0 = _base[:-1]
CORE_UN = [_base[k + 1] - _base[k] for k in range(NCORES)]

# main-kernel output layout (per core, in elements)
OFF_D1 = []
OFF_D2 = []
_off = 0
for _i in range(LK):
    _w = 3 ** (6 - _i)
    OFF_D1.append(_off)
    OFF_D2.append(_off + PAD_UNITS * _w)
    _off += 2 * PAD_UNITS * _w
OFF_F7 = _off
OUT_LEN = _off + PAD_UNITS


def _split_multi_waits(nc):
    """This walrus build rejects any instruction carrying >1 sync wait
    ("Too many sync wait commands").  Split extra waits onto single-wait
    NOPs inserted just before, on the same engine queue."""
    ctr = [0]
    for fn in nc.m.functions:
        for bb in fn.blocks:
            new = []
            for inst in bb.instructions:
                si = inst.sync_info
                if si is not None and si.on_wait and len(si.on_wait) > 1:
                    waits = list(si.on_wait)
                    for w in waits[:-1]:
                        ctr[0] += 1
                        new.append(mybir.InstNoOp(
                            name=f"splitw_{ctr[0]}",
                            engine=inst.engine,
                            bass_nofuse=True,
                            sync_info=mybir.SyncInfo(on_wait=[w], on_update=[]),
                        ))
                    si.on_wait = [waits[-1]]
                new.append(inst)
            bb.instructions = new


def _cols(phi_sb, pcol0):
    return (phi_sb[:, pcol0 + 0: pcol0 + 1],
            phi_sb[:, pcol0 + 1: pcol0 + 2],
            phi_sb[:, pcol0 + 2: pcol0 + 3])


def _thirds(src):
    W = src.shape[-1] // 3
    return src[:, 0:W], src[:, W:2 * W], src[:, 2 * W:3 * W]


def _sub(ap, col_off, dims):
    """Raw AP into an SBUF tile slice: same partition dim, custom free
    dims, offset by col_off columns."""
    return bass.AP(ap.tensor, ap.offset + col_off,
                   [list(ap.ap[0])] + [list(d) for d in dims])


def _avg_scatter(nc, dsts, tmp, src, phi_sb, pcol0, w):
    """avg = x0*c0 + x1*c1 + x2*c2 on DVE (packed ts/stt); the final
    fused op is emitted once per next-level plane i3, writing into
    dsts[i3] so the next level sees plane-major contiguous thirds.
    Each unit's w-wide avg block splits into 3 sub-blocks of w//3."""
    x0, x1, x2 = _thirds(src)
    c0, c1, c2 = _cols(phi_sb, pcol0)
    W = x0.shape[-1]
    nu = W // w                    # units covered (UPP or T*UPP)
    wn = w // 3
    nc.vector.tensor_scalar_mul(tmp[:, :W], x0, c0)
    nc.vector.scalar_tensor_tensor(tmp[:, :W], x1, c1, tmp[:, :W],
                                   MULT, ADD)
    for i3 in range(3):
        sx = _sub(x2, i3 * wn, [[w, nu], [1, wn]])
        sa = _sub(tmp[:, :W], i3 * wn, [[w, nu], [1, wn]])
        nc.vector.scalar_tensor_tensor(dsts[i3], sx, c2, sa, MULT, ADD)


def _det_branch(nc, dst, t1, t2, src, phi_sb, pcol0):
    """detail = x0*c0 + x1*c1 + x2*c2; ACT does two muls, DVE one packed
    mul and two packed adds."""
    x0, x1, x2 = _thirds(src)
    c0, c1, c2 = _cols(phi_sb, pcol0)
    W = x0.shape[-1]
    nc.vector.tensor_scalar_mul(dst, x0, c0)
    nc.scalar.mul(t1[:, :W], x1, c1)
    nc.scalar.mul(t2[:, :W], x2, c2)
    nc.vector.tensor_tensor(dst, dst, t1[:, :W], ADD)
    nc.vector.tensor_tensor(dst, dst, t2[:, :W], ADD)


def _sbranch(nc, dst, src, phi_sb, pcol0, first):
    """Strided (non-reversed) dot product for the tail: mul on `first`,
    both fused FMAs on DVE."""
    x0 = src[:, 0::3]
    x1 = src[:, 1::3]
    x2 = src[:, 2::3]
    c0, c1, c2 = _cols(phi_sb, pcol0)
    if first == "act":
        nc.scalar.mul(dst, x0, c0)
    else:
        nc.vector.tensor_scalar_mul(dst, x0, c0)
    nc.vector.scalar_tensor_tensor(dst, x1, c1, dst, MULT, ADD)
    nc.vector.scalar_tensor_tensor(dst, x2, c2, dst, MULT, ADD)


def _emit_tail(nc, pool, phi_sb, f7_tensor, f7_off, tail_out):
    """Levels 7..15 on the gathered f7 (19683 elems, unit order), bf16.
    avg chain stays on DVE; detail branches ride ACT+DVE."""
    X = pool.tile([81, 243], BF16, tag="X7", name="X7")
    nc.sync.dma_start(X[:], bass.AP(f7_tensor, f7_off, [[243, 81], [1, 243]]))

    def tail_level(cur, P, L):
        Wo = cur.shape[-1] // 3
        d1 = pool.tile([P, Wo], BF16, tag=f"td1_{L}", name=f"td1_{L}")
        d2 = pool.tile([P, Wo], BF16, tag=f"td2_{L}", name=f"td2_{L}")
        av = pool.tile([P, Wo], BF16, tag=f"ta_{L}", name=f"ta_{L}")
        pc = L * 9
        psb = phi_sb[0:P, :]
        _sbranch(nc, av[:], cur, psb, pc + 0, "dve")
        _sbranch(nc, d1[:], cur, psb, pc + 3, "act")
        _sbranch(nc, d2[:], cur, psb, pc + 6, "act")
        base = 3 ** (15 - L)
        for eng, dt_, off in ((nc.scalar, d1, base), (nc.sync, d2, 2 * base)):
            eng.dma_start(
                bass.AP(tail_out, off, [[Wo, P], [1, Wo]]), dt_[:])
        return av[:]

    cur = X[:]
    for L in range(LK, 12):                    # levels 7..11 on [81, W]
        cur = tail_level(cur, 81, L)
    # export the level-12 signal (81 values); host runs levels 12..15
    nc.sync.dma_start(bass.AP(tail_out, 0, [[1, 81], [1, 1]]), cur)


def build_main(nrep=1, in_bufs=2, copy_only=False, merge_tail=True,
               hoist_collective=False):
    """hoist_collective: timing-only variant — the AllGather runs once
    before the repeat loop (collectives inside For_i desync the mesh);
    everything else stays inside the loop."""
    nc = bass.Bass("TRN2", target_bir_lowering=False, debug=False,
                   num_devices=NCORES)
    x = nc.dram_tensor("x", [PAD_UNITS * UNIT], BF16, kind="ExternalInput")
    phi = nc.dram_tensor("phi", [128, NL * 9], F32, kind="ExternalInput")
    dg = nc.dram_tensor("dg", [128, 12 * 128], BF16, kind="ExternalInput")
    out = nc.dram_tensor("out", [OUT_LEN], BF16, kind="ExternalOutput")
    tail_out = (nc.dram_tensor("tail", [NUNITS], BF16,
                               kind="ExternalOutput")
                if merge_tail else None)

    FW = UPP * UNIT  # 4374 elems per partition per tile

    with tile.TileContext(nc) as tc:
        with (
            tc.tile_pool(name="phi_p", bufs=1) as phi_pool,
            tc.tile_pool(name="in_p", bufs=in_bufs) as in_pool,
            tc.tile_pool(name="a_p", bufs=2) as a_pool,
            tc.tile_pool(name="t_p", bufs=2) as t_pool,
            tc.tile_pool(name="st_p", bufs=1) as st_pool,
            tc.tile_pool(name="dd_p", bufs=1) as dd_pool,
            tc.tile_pool(name="r_p", bufs=1) as r_pool,
            tc.tile_pool(name="ps_p", bufs=1, space="PSUM") as ps_pool,
            tc.tile_pool(name="dram_p", bufs=1, space="DRAM") as dram_pool,
            tc.tile_pool(name="tail_p", bufs=1) as tail_pool,
        ):
            def dram_tiles():
                f7_loc = dram_pool.tile([PAD_UNITS], BF16, tag="f7_loc",
                                        name="f7_loc")
                f7_all = dram_pool.tile([NCORES * PAD_UNITS], BF16,
                                        tag="f7_all", name="f7_all",
                                        addr_space="Shared")
                # 8*2461 = 19688 >= NUNITS; last 5 slots = padding garbage
                f7_flat = dram_pool.tile([NCORES * 2461], BF16,
                                         tag="f7_flat", name="f7_flat")
                return f7_loc, f7_all, f7_flat

            def collective(f7_loc, f7_all):
                nc.gpsimd.collective_compute(
                    "AllGather",
                    mybir.AluOpType.bypass,
                    replica_groups=[list(range(NCORES))],
                    ins=[f7_loc.opt()],
                    outs=[f7_all.opt()],
                )

            def body():
                phi_sb = phi_pool.tile([128, NL * 9], F32, tag="phi",
                                       name="phi_sb")
                nc.sync.dma_start(phi_sb[:], phi[:])
                dgt = phi_pool.tile([128, 12 * 128], BF16, tag="dgt",
                                    name="dgt")
                nc.sync.dma_start(dgt[:], dg[:])

                # detail staging for levels 1..6 (+ f7 slab); layout per
                # partition: [tile t][unit u][w revpos] (t-major)
                ST = {}
                for lvl in range(1, LK):
                    w = 3 ** (6 - lvl)
                    for b in (1, 2):
                        ST[(lvl, b)] = st_pool.tile(
                            [128, T * UPP * w], BF16, tag=f"st{lvl}_{b}",
                            name=f"st{lvl}_{b}")
                # residents for batched levels 3..6, plane-major layout:
                # [plane i][tile t][unit u][w] so thirds stay contiguous
                R = {}
                for lvl in range(2, 7):
                    R[lvl] = r_pool.tile(
                        [128, T * UPP * 3 ** (6 - lvl) * 3], BF16,
                        tag=f"R{lvl}", name=f"R{lvl}")
                F7 = r_pool.tile([128, T * UPP], BF16, tag="F7", name="F7")

                # ---- streamed levels 0..2, one [128, 4374] tile each
                for t in range(T):
                    xt = in_pool.tile([128, FW], BF16, tag="xt", name="xt")
                    src = bass.AP(x, t * 128 * FW, [[FW, 128], [1, FW]])
                    nc.sync.dma_start(xt[:], src)

                    if copy_only:
                        dst_off = t * 128 * FW
                        n = min(FW, max(0, (OUT_LEN - dst_off) // 128))
                        if n > 0:
                            nc.sync.dma_start(
                                bass.AP(out, dst_off, [[n, 128], [1, n]]),
                                xt[:, :n])
                        continue

                    cur = xt[:]
                    for lvl in range(2):
                        w = 3 ** (6 - lvl)      # output width per unit
                        Wo = cur.shape[-1] // 3  # = UPP * w
                        d1 = (t_pool.tile([128, Wo], BF16, tag=f"d1_{lvl}",
                                          name=f"d1_{lvl}")[:]
                              if lvl == 0 else
                              ST[(lvl, 1)][:, t * Wo:(t + 1) * Wo])
                        d2 = (t_pool.tile([128, Wo], BF16, tag=f"d2_{lvl}",
                                          name=f"d2_{lvl}")[:]
                              if lvl == 0 else
                              ST[(lvl, 2)][:, t * Wo:(t + 1) * Wo])
                        t1 = t_pool.tile([128, Wo], BF16, tag=f"t1_{lvl}",
                                         name=f"t1_{lvl}")
                        wn = w // 3             # next-level width per unit
                        if lvl == 0:
                            av = a_pool.tile([128, Wo], BF16, tag=f"a{lvl}",
                                             name=f"a{lvl}")
                            dsts = [_sub(av[:], i3 * UPP * wn,
                                         [[wn, UPP], [1, wn]])
                                    for i3 in range(3)]
                            nxt = av[:]
                        else:
                            dsts = [_sub(R[2][:],
                                         i3 * T * UPP * wn + t * UPP * wn,
                                         [[wn, UPP], [1, wn]])
                                    for i3 in range(3)]
                            nxt = None
                        pc = lvl * 9
                        _avg_scatter(nc, dsts, t1, cur, phi_sb, pc + 0, w)
                        # detail branches on PE: 3 accumulating diag
                        # matmuls per 486-col chunk, j-outer so each
                        # weight loads once per (tile, branch); ACT casts
                        # psum fp32 -> SBUF bf16.
                        NCH = Wo // 486
                        for ri, dt_ in ((0, d1), (1, d2)):
                            # level 1 gets its own psum slots (ps6/ps7) so
                            # its matmuls don't wait on level-0 copy drains
                            s0 = 0 if lvl == 0 else 6
                            ps = [ps_pool.tile([128, 486], F32,
                                               tag=f"ps{s0 + c}",
                                               name=f"ps{s0 + c}")
                                  for c in range(NCH)]
                            for j in range(3):
                                bi = (lvl * 6 + ri * 3 + j) * 128
                                lhsT = dgt[:, bi: bi + 128]
                                for c in range(NCH):
                                    rhs = cur[:, j * Wo + c * 486:
                                              j * Wo + (c + 1) * 486]
                                    nc.tensor.matmul(
                                        ps[c][:], lhsT, rhs,
                                        start=(j == 0), stop=(j == 2))
                            for c in range(NCH):
                                dstv = dt_[:, c * 486:(c + 1) * 486]
                                if (c + ri) % 2 == 0:
                                    nc.scalar.copy(dstv, ps[c][:])
                                else:
                                    nc.vector.tensor_scalar_mul(
                                        dstv, ps[c][:], 1.0)
                        if lvl == 0:
                            for eng, dt_, off in ((nc.scalar, d1, OFF_D1[0]),
                                                  (nc.scalar, d2, OFF_D2[0])):
                                dst = bass.AP(out, off + t * 128 * Wo,
                                              [[Wo, 128], [1, Wo]])
                                eng.dma_start(dst, dt_)
                        cur = nxt

                if copy_only:
                    return

                # staged detail DMAs for level 1 (one per branch)
                for lvl in (1,):
                    for b, off in ((1, OFF_D1[lvl]), (2, OFF_D2[lvl])):
                        st = ST[(lvl, b)]
                        n = st.shape[-1]
                        nc.scalar.dma_start(
                            bass.AP(out, off, [[n, 128], [1, n]]), st[:])

                # ---- batched levels 2..6 over the whole resident buffer
                for lvl in range(2, LK):
                    w = 3 ** (6 - lvl)
                    cur = R[lvl][:]
                    Wo = cur.shape[-1] // 3     # T*UPP*w
                    wn = w // 3
                    d1 = ST[(lvl, 1)][:]
                    d2 = ST[(lvl, 2)][:]
                    t1 = dd_pool.tile([128, Wo], BF16, tag=f"bt1_{lvl}",
                                      name=f"bt1_{lvl}")
                    t2 = dd_pool.tile([128, Wo], BF16, tag=f"bt2_{lvl}",
                                      name=f"bt2_{lvl}")
                    t3 = dd_pool.tile([128, Wo], BF16, tag=f"bt3_{lvl}",
                                      name=f"bt3_{lvl}")
                    t4 = dd_pool.tile([128, Wo], BF16, tag=f"bt4_{lvl}",
                                      name=f"bt4_{lvl}")
                    t5 = dd_pool.tile([128, Wo], BF16, tag=f"bt5_{lvl}",
                                      name=f"bt5_{lvl}")
                    pc = lvl * 9
                    if lvl < 6:
                        # avg into R[lvl+1], plane-major: the i3 blocks of
                        # the next resident are contiguous (t,u,j) runs.
                        dsts = [_sub(R[lvl + 1][:], i3 * T * UPP * wn,
                                     [[wn, T * UPP], [1, wn]])
                                for i3 in range(3)]
                        _avg_scatter(nc, dsts, t1, cur, phi_sb, pc + 0, w)
                    else:
                        # level 6: avg -> F7 [128, (t,u)] dense, no reorder
                        x0, x1, x2 = _thirds(cur)
                        c0, c1, c2 = _cols(phi_sb, pc + 0)
                        nc.vector.tensor_scalar_mul(t1[:, :Wo], x0, c0)
                        nc.vector.scalar_tensor_tensor(
                            t1[:, :Wo], x1, c1, t1[:, :Wo], MULT, ADD)
                        nc.vector.scalar_tensor_tensor(
                            F7[:], x2, c2, t1[:, :Wo], MULT, ADD)
                    _det_branch(nc, d1, t2, t3, cur, phi_sb, pc + 3)
                    _det_branch(nc, d2, t4, t5, cur, phi_sb, pc + 6)
                    for b, off in ((1, OFF_D1[lvl]), (2, OFF_D2[lvl])):
                        st = ST[(lvl, b)]
                        n = st.shape[-1]
                        nc.scalar.dma_start(
                            bass.AP(out, off, [[n, 128], [1, n]]), st[:])

                # f7 slab out: value for unit g = t*256 + p*2 + u
                dstf = bass.AP(out, OFF_F7,
                               [[UPP, 128], [128 * UPP, T], [1, UPP]])
                nc.sync.dma_start(dstf, F7[:].rearrange("p (t j) -> p t j",
                                                        t=T))

                if merge_tail:
                    f7_loc, f7_all, f7_flat = dram_tiles()
                    nc.sync.dma_start(
                        bass.AP(f7_loc.tensor, f7_loc[:].offset,
                                [[UPP, 128], [128 * UPP, T], [1, UPP]]),
                        F7[:].rearrange("p (t j) -> p t j", t=T))
                    if not hoist_collective:
                        collective(f7_loc, f7_all)
                    # one strided DMA compacts padded slabs (U0[j]=j*2461)
                    nc.sync.dma_start(
                        f7_flat[:],
                        bass.AP(f7_all.tensor, f7_all[:].offset,
                                [[PAD_UNITS, NCORES], [1, 2461]]))
                    _emit_tail(nc, tail_pool, phi_sb, f7_flat.tensor,
                               f7_flat[:].offset, tail_out)

            if nrep == 1:
                body()
            else:
                if merge_tail and hoist_collective:
                    f7_loc, f7_all, _ = dram_tiles()
                    collective(f7_loc, f7_all)
                with tc.For_i(0, nrep, 1):
                    body()

    return nc


def _phi_from_inputs(Phi_P: np.ndarray) -> np.ndarray:
    Q = np.stack([np.linalg.qr(Phi_P[i].T.astype(np.float32))[0]
                  for i in range(Phi_P.shape[0])])
    return np.transpose(Q, (0, 2, 1)).astype(np.float32)


# ternary digit-reversal permutations: REV[k][i] = reverse of i in k digits
_REV = {}
for _k in range(1, 8):
    idx = np.arange(3 ** _k)
    r = np.zeros_like(idx)
    ii = idx.copy()
    for _d in range(_k):
        r = r * 3 + (ii % 3)
        ii //= 3
    _REV[_k] = r

_CACHE = {}


def _make_dg(Phi):
    """Diag weight blocks for the PE detail branches of levels 0..1."""
    dg = np.zeros((12, 128, 128), np.float32)
    for lvl in range(2):
        for ri, r in enumerate((1, 2)):
            for j in range(3):
                np.fill_diagonal(dg[lvl * 6 + ri * 3 + j], Phi[lvl, r, j])
    return dg.transpose(1, 0, 2).reshape(128, 12 * 128).astype(NPBF)


def kernel(f: np.ndarray, Phi_P: np.ndarray) -> np.ndarray:
    f = np.asarray(f, dtype=np.float32).ravel()
    Phi = _phi_from_inputs(np.asarray(Phi_P, dtype=np.float32))

    phi_all = np.broadcast_to(
        Phi.reshape(1, NL * 9), (128, NL * 9)).copy()
    dg = _make_dg(Phi)

    if "main" not in _CACHE:
        _CACHE["main"] = build_main()
        _split_multi_waits(_CACHE["main"])
    nc_main = _CACHE["main"]

    rev7 = _REV[7]
    in_maps = []
    for k in range(NCORES):
        lo = CORE_U0[k] * UNIT
        n = CORE_UN[k] * UNIT
        xk = np.zeros(PAD_UNITS * UNIT, dtype=np.float32)
        xk[:n] = f[lo:lo + n]
        # per unit: digit-reverse; per pair: plane-major [3][u][729]
        u = xk.reshape(PAD_UNITS // UPP, UPP, UNIT)[:, :, rev7]
        u = u.reshape(PAD_UNITS // UPP, UPP, 3, UNIT // 3)
        u = np.ascontiguousarray(u.transpose(0, 2, 1, 3))
        in_maps.append({"x": u.reshape(-1).astype(NPBF), "phi": phi_all,
                        "dg": dg})

    res = run_bass_kernel_spmd(nc_main, in_maps, list(range(NCORES)))

    f_hat = np.empty(3 ** NL, dtype=np.float32)
    for k in range(NCORES):
        ok = res.results[k]["out"]
        u0, un = CORE_U0[k], CORE_UN[k]
        for i in range(LK):
            w = 3 ** (6 - i)
            base = 3 ** (15 - i)
            for off, dst0 in ((OFF_D1[i], base), (OFF_D2[i], 2 * base)):
                blk = np.asarray(ok[off: off + PAD_UNITS * w])
                if i == 0:
                    # level 0 DMAs per tile: [t][p][u][w'] already g-major
                    blk = blk.reshape(PAD_UNITS, w)
                else:
                    # staged: [p][t][u][w'] -> [t][p][u][w'] = g-major
                    blk = blk.reshape(128, T, UPP, w).transpose(1, 0, 2, 3)
                    blk = blk.reshape(PAD_UNITS, w)
                if w > 1:
                    blk = blk[:, _REV[6 - i]]
                f_hat[dst0 + u0 * w: dst0 + (u0 + un) * w] = \
                    blk[:un].reshape(-1).astype(np.float32)
    f_hat[:NUNITS] = np.asarray(
        res.results[0]["tail"]).astype(np.float32)
    # device tail stops at level 12; finish levels 12..15 on host
    cur = f_hat[0:81].copy()
    for L in range(12, NL):
        fm = cur.reshape(-1, 3)
        base = 3 ** (15 - L)
        f_hat[base: base + fm.shape[0]] = fm @ Phi[L, 1, :]
        f_hat[2 * base: 2 * base + fm.shape[0]] = fm @ Phi[L, 2, :]
        cur = fm @ Phi[L, 0, :]
    f_hat[0] = cur[0]
    return f_hat


# revision 25
# speedup vs baseline: 1.0358x; 1.0358x over previous
"""Trainium2 Bass kernel for the 16-level ternary (Haar-style) wavelet
transform of f (len 3^16) with row-orthonormalized 3x3 Phi matrices.

Strategy:
  - Host: QR-orthonormalize the 3x3 Phi blocks; digit-reverse each
    3^7-element unit (ternary digit reversal) and lay each SBUF
    partition row out plane-major, so every level's three dot-product
    operands are contiguous thirds of the buffer.  That unlocks the
    DVE's packed bf16 modes (tensor_scalar ~3x, tensor_tensor ~1.7x)
    which strided access forfeits.
  - All signal data is bf16 (I/O DMA bytes halve); the 3x3 coefficients
    stay fp32 per-partition scalars.
  - Main SPMD kernel (8 cores): units of 3^7 = 2187 elems recurse
    levels 0..6 inside one partition; details stream straight out
    (levels 1+ via SBUF staging so the out-DMA count stays small).
  - Tail: level-7 signal f7 (19683 elems) is all-gathered in-NEFF and
    levels 7..15 run redundantly on every core (unit averages are
    order-independent, so digit reversal does not affect f7).
"""

import sys

for _p in ("/opt/trn_rl_repo",):
    if _p not in sys.path:
        sys.path.append(_p)

import numpy as np
import ml_dtypes

import concourse.bass as bass
import concourse.mybir as mybir
import concourse.tile as tile
from concourse.bass_utils import run_bass_kernel_spmd

F32 = mybir.dt.float32
BF16 = mybir.dt.bfloat16
NPBF = ml_dtypes.bfloat16
MULT = mybir.AluOpType.mult
ADD = mybir.AluOpType.add

NL = 16                   # total levels
LK = 7                    # levels computed by the main kernel (0..6)
UNIT = 3 ** LK            # 2187 input elems per unit
NUNITS = 3 ** (NL - LK)   # 19683 units overall
NCORES = 8
UPP = 4                   # units per partition per tile
T = 5                     # tiles per core
PAD_UNITS = T * 128 * UPP  # 2560 padded units per core

# contiguous unit ranges per core (2461 x7 + 2456)
_base = [0]
for _k in range(NCORES):
    _base.append(_base[-1] + (2461 if _k < 7 else NUNITS - 7 * 2461))
CORE_U0 = _base[:-1]
CORE_UN = [_base[k + 1] - _base[k] for k in range(NCORES)]

# main-kernel output layout (per core, in elements)
OFF_D1 = []
OFF_D2 = []
_off = 0
for _i in range(LK):
    _w = 3 ** (6 - _i)
    OFF_D1.append(_off)
    OFF_D2.append(_off + PAD_UNITS * _w)
    _off += 2 * PAD_UNITS * _w
OFF_F7 = _off
OUT_LEN = _off + PAD_UNITS


def _split_multi_waits(nc):
    """This walrus build rejects any instruction carrying >1 sync wait
    ("Too many sync wait commands").  Split extra waits onto single-wait
    NOPs inserted just before, on the same engine queue."""
    ctr = [0]
    for fn in nc.m.functions:
        for bb in fn.blocks:
            new = []
            for inst in bb.instructions:
                si = inst.sync_info
                if si is not None and si.on_wait and len(si.on_wait) > 1:
                    waits = list(si.on_wait)
                    for w in waits[:-1]:
                        ctr[0] += 1
                        new.append(mybir.InstNoOp(
                            name=f"splitw_{ctr[0]}",
                            engine=inst.engine,
                            bass_nofuse=True,
                            sync_info=mybir.SyncInfo(on_wait=[w], on_update=[]),
                        ))
                    si.on_wait = [waits[-1]]
                new.append(inst)
            bb.instructions = new


def _cols(phi_sb, pcol0):
    return (phi_sb[:, pcol0 + 0: pcol0 + 1],
            phi_sb[:, pcol0 + 1: pcol0 + 2],
            phi_sb[:, pcol0 + 2: pcol0 + 3])


def _thirds(src):
    W = src.shape[-1] // 3
    return src[:, 0:W], src[:, W:2 * W], src[:, 2 * W:3 * W]


def _sub(ap, col_off, dims):
    """Raw AP into an SBUF tile slice: same partition dim, custom free
    dims, offset by col_off columns."""
    return bass.AP(ap.tensor, ap.offset + col_off,
                   [list(ap.ap[0])] + [list(d) for d in dims])


def _avg_scatter(nc, dsts, tmp, src, phi_sb, pcol0, w):
    """avg = x0*c0 + x1*c1 + x2*c2 on DVE (packed ts/stt); the final
    fused op is emitted once per next-level plane i3, writing into
    dsts[i3] so the next level sees plane-major contiguous thirds.
    Each unit's w-wide avg block splits into 3 sub-blocks of w//3."""
    x0, x1, x2 = _thirds(src)
    c0, c1, c2 = _cols(phi_sb, pcol0)
    W = x0.shape[-1]
    nu = W // w                    # units covered (UPP or T*UPP)
    wn = w // 3
    nc.vector.tensor_scalar_mul(tmp[:, :W], x0, c0)
    nc.vector.scalar_tensor_tensor(tmp[:, :W], x1, c1, tmp[:, :W],
                                   MULT, ADD)
    for i3 in range(3):
        sx = _sub(x2, i3 * wn, [[w, nu], [1, wn]])
        sa = _sub(tmp[:, :W], i3 * wn, [[w, nu], [1, wn]])
        nc.vector.scalar_tensor_tensor(dsts[i3], sx, c2, sa, MULT, ADD)


def _det_branch(nc, dst, t1, t2, src, phi_sb, pcol0):
    """detail = x0*c0 + x1*c1 + x2*c2; ACT does two muls, DVE one packed
    mul and two packed adds."""
    x0, x1, x2 = _thirds(src)
    c0, c1, c2 = _cols(phi_sb, pcol0)
    W = x0.shape[-1]
    nc.vector.tensor_scalar_mul(dst, x0, c0)
    nc.scalar.mul(t1[:, :W], x1, c1)
    nc.scalar.mul(t2[:, :W], x2, c2)
    nc.vector.tensor_tensor(dst, dst, t1[:, :W], ADD)
    nc.vector.tensor_tensor(dst, dst, t2[:, :W], ADD)


def _sbranch(nc, dst, src, phi_sb, pcol0, first):
    """Strided (non-reversed) dot product for the tail: mul on `first`,
    both fused FMAs on DVE."""
    x0 = src[:, 0::3]
    x1 = src[:, 1::3]
    x2 = src[:, 2::3]
    c0, c1, c2 = _cols(phi_sb, pcol0)
    if first == "act":
        nc.scalar.mul(dst, x0, c0)
    else:
        nc.vector.tensor_scalar_mul(dst, x0, c0)
    nc.vector.scalar_tensor_tensor(dst, x1, c1, dst, MULT, ADD)
    nc.vector.scalar_tensor_tensor(dst, x2, c2, dst, MULT, ADD)


def _emit_tail(nc, pool, phi_sb, f7_tensor, f7_off, tail_out):
    """Levels 7..15 on the gathered f7 (19683 elems, unit order), bf16.
    avg chain stays on DVE; detail branches ride ACT+DVE."""
    X = pool.tile([81, 243], BF16, tag="X7", name="X7")
    nc.sync.dma_start(X[:], bass.AP(f7_tensor, f7_off, [[243, 81], [1, 243]]))

    def tail_level(cur, P, L):
        Wo = cur.shape[-1] // 3
        d1 = pool.tile([P, Wo], BF16, tag=f"td1_{L}", name=f"td1_{L}")
        d2 = pool.tile([P, Wo], BF16, tag=f"td2_{L}", name=f"td2_{L}")
        av = pool.tile([P, Wo], BF16, tag=f"ta_{L}", name=f"ta_{L}")
        pc = L * 9
        psb = phi_sb[0:P, :]
        _sbranch(nc, av[:], cur, psb, pc + 0, "dve")
        _sbranch(nc, d1[:], cur, psb, pc + 3, "act")
        _sbranch(nc, d2[:], cur, psb, pc + 6, "act")
        base = 3 ** (15 - L)
        for eng, dt_, off in ((nc.scalar, d1, base), (nc.sync, d2, 2 * base)):
            eng.dma_start(
                bass.AP(tail_out, off, [[Wo, P], [1, Wo]]), dt_[:])
        return av[:]

    cur = X[:]
    for L in range(LK, 12):                    # levels 7..11 on [81, W]
        cur = tail_level(cur, 81, L)
    # export the level-12 signal (81 values); host runs levels 12..15
    nc.sync.dma_start(bass.AP(tail_out, 0, [[1, 81], [1, 1]]), cur)


def build_main(nrep=1, in_bufs=3, copy_only=False, merge_tail=True,
               hoist_collective=False):
    """hoist_collective: timing-only variant — the AllGather runs once
    before the repeat loop (collectives inside For_i desync the mesh);
    everything else stays inside the loop."""
    nc = bass.Bass("TRN2", target_bir_lowering=False, debug=False,
                   num_devices=NCORES)
    x = nc.dram_tensor("x", [PAD_UNITS * UNIT], BF16, kind="ExternalInput")
    phi = nc.dram_tensor("phi", [128, NL * 9], F32, kind="ExternalInput")
    dg = nc.dram_tensor("dg", [128, 12 * 128], BF16, kind="ExternalInput")
    out = nc.dram_tensor("out", [OUT_LEN], BF16, kind="ExternalOutput")
    tail_out = (nc.dram_tensor("tail", [NUNITS], BF16,
                               kind="ExternalOutput")
                if merge_tail else None)

    FW = UPP * UNIT  # 4374 elems per partition per tile

    with tile.TileContext(nc) as tc:
        with (
            tc.tile_pool(name="phi_p", bufs=1) as phi_pool,
            tc.tile_pool(name="in_p", bufs=in_bufs) as in_pool,
            tc.tile_pool(name="a_p", bufs=3) as a_pool,
            tc.tile_pool(name="t_p", bufs=3) as t_pool,
            tc.tile_pool(name="st_p", bufs=1) as st_pool,
            tc.tile_pool(name="dd_p", bufs=1) as dd_pool,
            tc.tile_pool(name="r_p", bufs=1) as r_pool,
            tc.tile_pool(name="ps_p", bufs=1, space="PSUM") as ps_pool,
            tc.tile_pool(name="dram_p", bufs=1, space="DRAM") as dram_pool,
            tc.tile_pool(name="tail_p", bufs=1) as tail_pool,
        ):
            def dram_tiles():
                f7_loc = dram_pool.tile([PAD_UNITS], BF16, tag="f7_loc",
                                        name="f7_loc")
                f7_all = dram_pool.tile([NCORES * PAD_UNITS], BF16,
                                        tag="f7_all", name="f7_all",
                                        addr_space="Shared")
                # 8*2461 = 19688 >= NUNITS; last 5 slots = padding garbage
                f7_flat = dram_pool.tile([NCORES * 2461], BF16,
                                         tag="f7_flat", name="f7_flat")
                return f7_loc, f7_all, f7_flat

            def collective(f7_loc, f7_all):
                nc.gpsimd.collective_compute(
                    "AllGather",
                    mybir.AluOpType.bypass,
                    replica_groups=[list(range(NCORES))],
                    ins=[f7_loc.opt()],
                    outs=[f7_all.opt()],
                )

            def body():
                phi_sb = phi_pool.tile([128, NL * 9], F32, tag="phi",
                                       name="phi_sb")
                nc.sync.dma_start(phi_sb[:], phi[:])
                dgt = phi_pool.tile([128, 12 * 128], BF16, tag="dgt",
                                    name="dgt")
                nc.sync.dma_start(dgt[:], dg[:])

                # detail staging for levels 1..6 (+ f7 slab); layout per
                # partition: [tile t][unit u][w revpos] (t-major)
                ST = {}
                for lvl in range(1, LK):
                    w = 3 ** (6 - lvl)
                    for b in (1, 2):
                        ST[(lvl, b)] = st_pool.tile(
                            [128, T * UPP * w], BF16, tag=f"st{lvl}_{b}",
                            name=f"st{lvl}_{b}")
                # residents for batched levels 3..6, plane-major layout:
                # [plane i][tile t][unit u][w] so thirds stay contiguous
                R = {}
                for lvl in range(2, 7):
                    R[lvl] = r_pool.tile(
                        [128, T * UPP * 3 ** (6 - lvl) * 3], BF16,
                        tag=f"R{lvl}", name=f"R{lvl}")
                F7 = r_pool.tile([128, T * UPP], BF16, tag="F7", name="F7")

                # ---- streamed levels 0..2, one [128, 4374] tile each
                for t in range(T):
                    xt = in_pool.tile([128, FW], BF16, tag="xt", name="xt")
                    src = bass.AP(x, t * 128 * FW, [[FW, 128], [1, FW]])
                    nc.sync.dma_start(xt[:], src)

                    if copy_only:
                        dst_off = t * 128 * FW
                        n = min(FW, max(0, (OUT_LEN - dst_off) // 128))
                        if n > 0:
                            nc.sync.dma_start(
                                bass.AP(out, dst_off, [[n, 128], [1, n]]),
                                xt[:, :n])
                        continue

                    cur = xt[:]
                    for lvl in range(2):
                        w = 3 ** (6 - lvl)      # output width per unit
                        Wo = cur.shape[-1] // 3  # = UPP * w
                        d1 = (t_pool.tile([128, Wo], BF16, tag=f"d1_{lvl}",
                                          name=f"d1_{lvl}")[:]
                              if lvl == 0 else
                              ST[(lvl, 1)][:, t * Wo:(t + 1) * Wo])
                        d2 = (t_pool.tile([128, Wo], BF16, tag=f"d2_{lvl}",
                                          name=f"d2_{lvl}")[:]
                              if lvl == 0 else
                              ST[(lvl, 2)][:, t * Wo:(t + 1) * Wo])
                        t1 = t_pool.tile([128, Wo], BF16, tag=f"t1_{lvl}",
                                         name=f"t1_{lvl}")
                        wn = w // 3             # next-level width per unit
                        if lvl == 0:
                            av = a_pool.tile([128, Wo], BF16, tag=f"a{lvl}",
                                             name=f"a{lvl}")
                            dsts = [_sub(av[:], i3 * UPP * wn,
                                         [[wn, UPP], [1, wn]])
                                    for i3 in range(3)]
                            nxt = av[:]
                        else:
                            dsts = [_sub(R[2][:],
                                         i3 * T * UPP * wn + t * UPP * wn,
                                         [[wn, UPP], [1, wn]])
                                    for i3 in range(3)]
                            nxt = None
                        pc = lvl * 9
                        _avg_scatter(nc, dsts, t1, cur, phi_sb, pc + 0, w)
                        # detail branches on PE: 3 accumulating diag
                        # matmuls per 486-col chunk, j-outer so each
                        # weight loads once per (tile, branch); ACT casts
                        # psum fp32 -> SBUF bf16.
                        NCH = Wo // 486
                        for ri, dt_ in ((0, d1), (1, d2)):
                            # level 1 gets its own psum slots (ps6/ps7) so
                            # its matmuls don't wait on level-0 copy drains
                            s0 = 0 if lvl == 0 else 6
                            ps = [ps_pool.tile([128, 486], F32,
                                               tag=f"ps{s0 + c}",
                                               name=f"ps{s0 + c}")
                                  for c in range(NCH)]
                            for j in range(3):
                                bi = (lvl * 6 + ri * 3 + j) * 128
                                lhsT = dgt[:, bi: bi + 128]
                                for c in range(NCH):
                                    rhs = cur[:, j * Wo + c * 486:
                                              j * Wo + (c + 1) * 486]
                                    nc.tensor.matmul(
                                        ps[c][:], lhsT, rhs,
                                        start=(j == 0), stop=(j == 2))
                            for c in range(NCH):
                                dstv = dt_[:, c * 486:(c + 1) * 486]
                                if (c + ri) % 2 == 0:
                                    nc.scalar.copy(dstv, ps[c][:])
                                else:
                                    nc.vector.tensor_scalar_mul(
                                        dstv, ps[c][:], 1.0)
                        if lvl == 0:
                            for eng, dt_, off in ((nc.scalar, d1, OFF_D1[0]),
                                                  (nc.sync, d2, OFF_D2[0])):
                                dst = bass.AP(out, off + t * 128 * Wo,
                                              [[Wo, 128], [1, Wo]])
                                eng.dma_start(dst, dt_)
                        cur = nxt

                if copy_only:
                    return

                # staged detail DMAs for level 1 (one per branch)
                for lvl in (1,):
                    for b, off in ((1, OFF_D1[lvl]), (2, OFF_D2[lvl])):
                        st = ST[(lvl, b)]
                        n = st.shape[-1]
                        nc.sync.dma_start(
                            bass.AP(out, off, [[n, 128], [1, n]]), st[:])

                # ---- batched levels 2..6 over the whole resident buffer
                for lvl in range(2, LK):
                    w = 3 ** (6 - lvl)
                    cur = R[lvl][:]
                    Wo = cur.shape[-1] // 3     # T*UPP*w
                    wn = w // 3
                    d1 = ST[(lvl, 1)][:]
                    d2 = ST[(lvl, 2)][:]
                    t1 = dd_pool.tile([128, Wo], BF16, tag=f"bt1_{lvl}",
                                      name=f"bt1_{lvl}")
                    t2 = dd_pool.tile([128, Wo], BF16, tag=f"bt2_{lvl}",
                                      name=f"bt2_{lvl}")
                    t3 = dd_pool.tile([128, Wo], BF16, tag=f"bt3_{lvl}",
                                      name=f"bt3_{lvl}")
                    t4 = dd_pool.tile([128, Wo], BF16, tag=f"bt4_{lvl}",
                                      name=f"bt4_{lvl}")
                    t5 = dd_pool.tile([128, Wo], BF16, tag=f"bt5_{lvl}",
                                      name=f"bt5_{lvl}")
                    pc = lvl * 9
                    if lvl < 6:
                        # avg into R[lvl+1], plane-major: the i3 blocks of
                        # the next resident are contiguous (t,u,j) runs.
                        dsts = [_sub(R[lvl + 1][:], i3 * T * UPP * wn,
                                     [[wn, T * UPP], [1, wn]])
                                for i3 in range(3)]
                        _avg_scatter(nc, dsts, t1, cur, phi_sb, pc + 0, w)
                    else:
                        # level 6: avg -> F7 [128, (t,u)] dense, no reorder
                        x0, x1, x2 = _thirds(cur)
                        c0, c1, c2 = _cols(phi_sb, pc + 0)
                        nc.vector.tensor_scalar_mul(t1[:, :Wo], x0, c0)
                        nc.vector.scalar_tensor_tensor(
                            t1[:, :Wo], x1, c1, t1[:, :Wo], MULT, ADD)
                        nc.vector.scalar_tensor_tensor(
                            F7[:], x2, c2, t1[:, :Wo], MULT, ADD)
                    _det_branch(nc, d1, t2, t3, cur, phi_sb, pc + 3)
                    _det_branch(nc, d2, t4, t5, cur, phi_sb, pc + 6)
                    for b, off in ((1, OFF_D1[lvl]), (2, OFF_D2[lvl])):
                        st = ST[(lvl, b)]
                        n = st.shape[-1]
                        nc.scalar.dma_start(
                            bass.AP(out, off, [[n, 128], [1, n]]), st[:])

                # f7 slab out: value for unit g = t*256 + p*2 + u
                dstf = bass.AP(out, OFF_F7,
                               [[UPP, 128], [128 * UPP, T], [1, UPP]])
                nc.sync.dma_start(dstf, F7[:].rearrange("p (t j) -> p t j",
                                                        t=T))

                if merge_tail:
                    f7_loc, f7_all, f7_flat = dram_tiles()
                    nc.sync.dma_start(
                        bass.AP(f7_loc.tensor, f7_loc[:].offset,
                                [[UPP, 128], [128 * UPP, T], [1, UPP]]),
                        F7[:].rearrange("p (t j) -> p t j", t=T))
                    if not hoist_collective:
                        collective(f7_loc, f7_all)
                    # one strided DMA compacts padded slabs (U0[j]=j*2461)
                    nc.sync.dma_start(
                        f7_flat[:],
                        bass.AP(f7_all.tensor, f7_all[:].offset,
                                [[PAD_UNITS, NCORES], [1, 2461]]))
                    _emit_tail(nc, tail_pool, phi_sb, f7_flat.tensor,
                               f7_flat[:].offset, tail_out)

            if nrep == 1:
                body()
            else:
                if merge_tail and hoist_collective:
                    f7_loc, f7_all, _ = dram_tiles()
                    collective(f7_loc, f7_all)
                with tc.For_i(0, nrep, 1):
                    body()

    return nc


def _phi_from_inputs(Phi_P: np.ndarray) -> np.ndarray:
    Q = np.stack([np.linalg.qr(Phi_P[i].T.astype(np.float32))[0]
                  for i in range(Phi_P.shape[0])])
    return np.transpose(Q, (0, 2, 1)).astype(np.float32)


# ternary digit-reversal permutations: REV[k][i] = reverse of i in k digits
_REV = {}
for _k in range(1, 8):
    idx = np.arange(3 ** _k)
    r = np.zeros_like(idx)
    ii = idx.copy()
    for _d in range(_k):
        r = r * 3 + (ii % 3)
        ii //= 3
    _REV[_k] = r

_CACHE = {}


def _make_dg(Phi):
    """Diag weight blocks for the PE detail branches of levels 0..1."""
    dg = np.zeros((12, 128, 128), np.float32)
    for lvl in range(2):
        for ri, r in enumerate((1, 2)):
            for j in range(3):
                np.fill_diagonal(dg[lvl * 6 + ri * 3 + j], Phi[lvl, r, j])
    return dg.transpose(1, 0, 2).reshape(128, 12 * 128).astype(NPBF)


def kernel(f: np.ndarray, Phi_P: np.ndarray) -> np.ndarray:
    f = np.asarray(f, dtype=np.float32).ravel()
    Phi = _phi_from_inputs(np.asarray(Phi_P, dtype=np.float32))

    phi_all = np.broadcast_to(
        Phi.reshape(1, NL * 9), (128, NL * 9)).copy()
    dg = _make_dg(Phi)

    if "main" not in _CACHE:
        _CACHE["main"] = build_main()
        _split_multi_waits(_CACHE["main"])
    nc_main = _CACHE["main"]

    rev7 = _REV[7]
    in_maps = []
    for k in range(NCORES):
        lo = CORE_U0[k] * UNIT
        n = CORE_UN[k] * UNIT
        xk = np.zeros(PAD_UNITS * UNIT, dtype=np.float32)
        xk[:n] = f[lo:lo + n]
        # per unit: digit-reverse; per pair: plane-major [3][u][729]
        u = xk.reshape(PAD_UNITS // UPP, UPP, UNIT)[:, :, rev7]
        u = u.reshape(PAD_UNITS // UPP, UPP, 3, UNIT // 3)
        u = np.ascontiguousarray(u.transpose(0, 2, 1, 3))
        in_maps.append({"x": u.reshape(-1).astype(NPBF), "phi": phi_all,
                        "dg": dg})

    res = run_bass_kernel_spmd(nc_main, in_maps, list(range(NCORES)))

    f_hat = np.empty(3 ** NL, dtype=np.float32)
    for k in range(NCORES):
        ok = res.results[k]["out"]
        u0, un = CORE_U0[k], CORE_UN[k]
        for i in range(LK):
            w = 3 ** (6 - i)
            base = 3 ** (15 - i)
            for off, dst0 in ((OFF_D1[i], base), (OFF_D2[i], 2 * base)):
                blk = np.asarray(ok[off: off + PAD_UNITS * w])
                if i == 0:
                    # level 0 DMAs per tile: [t][p][u][w'] already g-major
                    blk = blk.reshape(PAD_UNITS, w)
                else:
                    # staged: [p][t][u][w'] -> [t][p][u][w'] = g-major
                    blk = blk.reshape(128, T, UPP, w).transpose(1, 0, 2, 3)
                    blk = blk.reshape(PAD_UNITS, w)
                if w > 1:
                    blk = blk[:, _REV[6 - i]]
                f_hat[dst0 + u0 * w: dst0 + (u0 + un) * w] = \
                    blk[:un].reshape(-1).astype(np.float32)
    f_hat[:NUNITS] = np.asarray(
        res.results[0]["tail"]).astype(np.float32)
    # device tail stops at level 12; finish levels 12..15 on host
    cur = f_hat[0:81].copy()
    for L in range(12, NL):
        fm = cur.reshape(-1, 3)
        base = 3 ** (15 - L)
        f_hat[base: base + fm.shape[0]] = fm @ Phi[L, 1, :]
        f_hat[2 * base: 2 * base + fm.shape[0]] = fm @ Phi[L, 2, :]
        cur = fm @ Phi[L, 0, :]
    f_hat[0] = cur[0]
    return f_hat


# revision 27
# speedup vs baseline: 1.1001x; 1.0621x over previous
"""Trainium2 Bass kernel for the 16-level ternary (Haar-style) wavelet
transform of f (len 3^16) with row-orthonormalized 3x3 Phi matrices.

Strategy:
  - Host: QR-orthonormalize the 3x3 Phi blocks; digit-reverse each
    3^7-element unit (ternary digit reversal) and lay each SBUF
    partition row out plane-major, so every level's three dot-product
    operands are contiguous thirds of the buffer.  That unlocks the
    DVE's packed bf16 modes (tensor_scalar ~3x, tensor_tensor ~1.7x)
    which strided access forfeits.
  - All signal data is bf16 (I/O DMA bytes halve); the 3x3 coefficients
    stay fp32 per-partition scalars.
  - Main SPMD kernel (8 cores): units of 3^7 = 2187 elems recurse
    levels 0..6 inside one partition; details stream straight out
    (levels 1+ via SBUF staging so the out-DMA count stays small).
  - Tail: level-7 signal f7 (19683 elems) is all-gathered in-NEFF and
    levels 7..15 run redundantly on every core (unit averages are
    order-independent, so digit reversal does not affect f7).
"""

import sys

for _p in ("/opt/trn_rl_repo",):
    if _p not in sys.path:
        sys.path.append(_p)

import numpy as np
import ml_dtypes

import concourse.bass as bass
import concourse.mybir as mybir
import concourse.tile as tile
from concourse.bass_utils import run_bass_kernel_spmd

F32 = mybir.dt.float32
BF16 = mybir.dt.bfloat16
NPBF = ml_dtypes.bfloat16
MULT = mybir.AluOpType.mult
ADD = mybir.AluOpType.add

NL = 16                   # total levels
LK = 7                    # levels computed by the main kernel (0..6)
UNIT = 3 ** LK            # 2187 input elems per unit
NUNITS = 3 ** (NL - LK)   # 19683 units overall
NCORES = 8
UPP = 4                   # units per partition per tile
T = 5                     # tiles per core
PAD_UNITS = T * 128 * UPP  # 2560 padded units per core

# contiguous unit ranges per core (2461 x7 + 2456)
_base = [0]
for _k in range(NCORES):
    _base.append(_base[-1] + (2461 if _k < 7 else NUNITS - 7 * 2461))
CORE_U0 = _base[:-1]
CORE_UN = [_base[k + 1] - _base[k] for k in range(NCORES)]

# main-kernel output layout (per core, in elements)
OFF_D1 = []
OFF_D2 = []
_off = 0
for _i in range(LK):
    _w = 3 ** (6 - _i)
    OFF_D1.append(_off)
    OFF_D2.append(_off + PAD_UNITS * _w)
    _off += 2 * PAD_UNITS * _w
OFF_F7 = _off
OUT_LEN = _off + PAD_UNITS


def _split_multi_waits(nc):
    """This walrus build rejects any instruction carrying >1 sync wait
    ("Too many sync wait commands").  Split extra waits onto single-wait
    NOPs inserted just before, on the same engine queue."""
    ctr = [0]
    for fn in nc.m.functions:
        for bb in fn.blocks:
            new = []
            for inst in bb.instructions:
                si = inst.sync_info
                if si is not None and si.on_wait and len(si.on_wait) > 1:
                    waits = list(si.on_wait)
                    for w in waits[:-1]:
                        ctr[0] += 1
                        new.append(mybir.InstNoOp(
                            name=f"splitw_{ctr[0]}",
                            engine=inst.engine,
                            bass_nofuse=True,
                            sync_info=mybir.SyncInfo(on_wait=[w], on_update=[]),
                        ))
                    si.on_wait = [waits[-1]]
                new.append(inst)
            bb.instructions = new


def _cols(phi_sb, pcol0):
    return (phi_sb[:, pcol0 + 0: pcol0 + 1],
            phi_sb[:, pcol0 + 1: pcol0 + 2],
            phi_sb[:, pcol0 + 2: pcol0 + 3])


def _thirds(src):
    W = src.shape[-1] // 3
    return src[:, 0:W], src[:, W:2 * W], src[:, 2 * W:3 * W]


def _sub(ap, col_off, dims):
    """Raw AP into an SBUF tile slice: same partition dim, custom free
    dims, offset by col_off columns."""
    return bass.AP(ap.tensor, ap.offset + col_off,
                   [list(ap.ap[0])] + [list(d) for d in dims])


def _avg_scatter(nc, dsts, tmp, src, phi_sb, pcol0, w):
    """avg = x0*c0 + x1*c1 + x2*c2 on DVE (packed ts/stt); the final
    fused op is emitted once per next-level plane i3, writing into
    dsts[i3] so the next level sees plane-major contiguous thirds.
    Each unit's w-wide avg block splits into 3 sub-blocks of w//3."""
    x0, x1, x2 = _thirds(src)
    c0, c1, c2 = _cols(phi_sb, pcol0)
    W = x0.shape[-1]
    nu = W // w                    # units covered (UPP or T*UPP)
    wn = w // 3
    nc.vector.tensor_scalar_mul(tmp[:, :W], x0, c0)
    nc.vector.scalar_tensor_tensor(tmp[:, :W], x1, c1, tmp[:, :W],
                                   MULT, ADD)
    for i3 in range(3):
        sx = _sub(x2, i3 * wn, [[w, nu], [1, wn]])
        sa = _sub(tmp[:, :W], i3 * wn, [[w, nu], [1, wn]])
        nc.vector.scalar_tensor_tensor(dsts[i3], sx, c2, sa, MULT, ADD)


def _det_branch(nc, dst, t1, t2, src, phi_sb, pcol0):
    """detail = x0*c0 + x1*c1 + x2*c2; ACT does two muls, DVE one packed
    mul and two packed adds."""
    x0, x1, x2 = _thirds(src)
    c0, c1, c2 = _cols(phi_sb, pcol0)
    W = x0.shape[-1]
    nc.vector.tensor_scalar_mul(dst, x0, c0)
    nc.scalar.mul(t1[:, :W], x1, c1)
    nc.scalar.mul(t2[:, :W], x2, c2)
    nc.vector.tensor_tensor(dst, dst, t1[:, :W], ADD)
    nc.vector.tensor_tensor(dst, dst, t2[:, :W], ADD)


def _sbranch(nc, dst, src, phi_sb, pcol0, first):
    """Strided (non-reversed) dot product for the tail: mul on `first`,
    both fused FMAs on DVE."""
    x0 = src[:, 0::3]
    x1 = src[:, 1::3]
    x2 = src[:, 2::3]
    c0, c1, c2 = _cols(phi_sb, pcol0)
    if first == "act":
        nc.scalar.mul(dst, x0, c0)
    else:
        nc.vector.tensor_scalar_mul(dst, x0, c0)
    nc.vector.scalar_tensor_tensor(dst, x1, c1, dst, MULT, ADD)
    nc.vector.scalar_tensor_tensor(dst, x2, c2, dst, MULT, ADD)


def _emit_tail(nc, pool, phi_sb, f7_tensor, f7_off, tail_out):
    """Levels 7..15 on the gathered f7 (19683 elems, unit order), bf16.
    avg chain stays on DVE; detail branches ride ACT+DVE."""
    X = pool.tile([81, 243], BF16, tag="X7", name="X7")
    nc.sync.dma_start(X[:], bass.AP(f7_tensor, f7_off, [[243, 81], [1, 243]]))

    def tail_level(cur, P, L):
        Wo = cur.shape[-1] // 3
        d1 = pool.tile([P, Wo], BF16, tag=f"td1_{L}", name=f"td1_{L}")
        d2 = pool.tile([P, Wo], BF16, tag=f"td2_{L}", name=f"td2_{L}")
        av = pool.tile([P, Wo], BF16, tag=f"ta_{L}", name=f"ta_{L}")
        pc = L * 9
        psb = phi_sb[0:P, :]
        _sbranch(nc, av[:], cur, psb, pc + 0, "dve")
        _sbranch(nc, d1[:], cur, psb, pc + 3, "act")
        _sbranch(nc, d2[:], cur, psb, pc + 6, "act")
        base = 3 ** (15 - L)
        for eng, dt_, off in ((nc.scalar, d1, base), (nc.sync, d2, 2 * base)):
            eng.dma_start(
                bass.AP(tail_out, off, [[Wo, P], [1, Wo]]), dt_[:])
        return av[:]

    cur = X[:]
    for L in range(LK, 12):                    # levels 7..11 on [81, W]
        cur = tail_level(cur, 81, L)
    # export the level-12 signal (81 values); host runs levels 12..15
    nc.sync.dma_start(bass.AP(tail_out, 0, [[1, 81], [1, 1]]), cur)


def build_main(nrep=1, in_bufs=2, copy_only=False, merge_tail=True,
               hoist_collective=False):
    """hoist_collective: timing-only variant — the AllGather runs once
    before the repeat loop (collectives inside For_i desync the mesh);
    everything else stays inside the loop."""
    nc = bass.Bass("TRN2", target_bir_lowering=False, debug=False,
                   num_devices=NCORES)
    x = nc.dram_tensor("x", [PAD_UNITS * UNIT], BF16, kind="ExternalInput")
    phi = nc.dram_tensor("phi", [128, NL * 9], F32, kind="ExternalInput")
    dg = nc.dram_tensor("dg", [128, 12 * 128], BF16, kind="ExternalInput")
    out = nc.dram_tensor("out", [OUT_LEN], BF16, kind="ExternalOutput")
    tail_out = (nc.dram_tensor("tail", [NUNITS], BF16,
                               kind="ExternalOutput")
                if merge_tail else None)

    FW = UPP * UNIT  # 4374 elems per partition per tile

    with tile.TileContext(nc) as tc:
        with (
            tc.tile_pool(name="phi_p", bufs=1) as phi_pool,
            tc.tile_pool(name="in_p", bufs=in_bufs) as in_pool,
            tc.tile_pool(name="a_p", bufs=2) as a_pool,
            tc.tile_pool(name="t_p", bufs=2) as t_pool,
            tc.tile_pool(name="st_p", bufs=1) as st_pool,
            tc.tile_pool(name="dd_p", bufs=1) as dd_pool,
            tc.tile_pool(name="r_p", bufs=1) as r_pool,
            tc.tile_pool(name="ps_p", bufs=1, space="PSUM") as ps_pool,
            tc.tile_pool(name="dram_p", bufs=1, space="DRAM") as dram_pool,
            tc.tile_pool(name="tail_p", bufs=1) as tail_pool,
        ):
            def dram_tiles():
                f7_loc = dram_pool.tile([PAD_UNITS], BF16, tag="f7_loc",
                                        name="f7_loc")
                f7_all = dram_pool.tile([NCORES * PAD_UNITS], BF16,
                                        tag="f7_all", name="f7_all",
                                        addr_space="Shared")
                # 8*2461 = 19688 >= NUNITS; last 5 slots = padding garbage
                f7_flat = dram_pool.tile([NCORES * 2461], BF16,
                                         tag="f7_flat", name="f7_flat")
                return f7_loc, f7_all, f7_flat

            def collective(f7_loc, f7_all):
                nc.gpsimd.collective_compute(
                    "AllGather",
                    mybir.AluOpType.bypass,
                    replica_groups=[list(range(NCORES))],
                    ins=[f7_loc.opt()],
                    outs=[f7_all.opt()],
                )

            def body():
                phi_sb = phi_pool.tile([128, NL * 9], F32, tag="phi",
                                       name="phi_sb")
                nc.sync.dma_start(phi_sb[:], phi[:])
                dgt = phi_pool.tile([128, 12 * 128], BF16, tag="dgt",
                                    name="dgt")
                nc.sync.dma_start(dgt[:], dg[:])

                # detail staging for levels 1..6 (+ f7 slab); layout per
                # partition: [tile t][unit u][w revpos] (t-major)
                ST = {}
                for lvl in range(1, LK):
                    w = 3 ** (6 - lvl)
                    for b in (1, 2):
                        ST[(lvl, b)] = st_pool.tile(
                            [128, T * UPP * w], BF16, tag=f"st{lvl}_{b}",
                            name=f"st{lvl}_{b}")
                # residents for batched levels 3..6, plane-major layout:
                # [plane i][tile t][unit u][w] so thirds stay contiguous
                R = {}
                for lvl in range(2, 7):
                    R[lvl] = r_pool.tile(
                        [128, T * UPP * 3 ** (6 - lvl) * 3], BF16,
                        tag=f"R{lvl}", name=f"R{lvl}")
                F7 = r_pool.tile([128, T * UPP], BF16, tag="F7", name="F7")

                # ---- streamed levels 0..2, one [128, 4374] tile each
                for t in range(T):
                    xt = in_pool.tile([128, FW], BF16, tag="xt", name="xt")
                    src = bass.AP(x, t * 128 * FW, [[FW, 128], [1, FW]])
                    nc.sync.dma_start(xt[:], src)

                    if copy_only:
                        dst_off = t * 128 * FW
                        n = min(FW, max(0, (OUT_LEN - dst_off) // 128))
                        if n > 0:
                            nc.sync.dma_start(
                                bass.AP(out, dst_off, [[n, 128], [1, n]]),
                                xt[:, :n])
                        continue

                    cur = xt[:]
                    for lvl in range(2):
                        w = 3 ** (6 - lvl)      # output width per unit
                        Wo = cur.shape[-1] // 3  # = UPP * w
                        d1 = (t_pool.tile([128, Wo], BF16, tag=f"d1_{lvl}",
                                          name=f"d1_{lvl}")[:]
                              if lvl == 0 else
                              ST[(lvl, 1)][:, t * Wo:(t + 1) * Wo])
                        d2 = (t_pool.tile([128, Wo], BF16, tag=f"d2_{lvl}",
                                          name=f"d2_{lvl}")[:]
                              if lvl == 0 else
                              ST[(lvl, 2)][:, t * Wo:(t + 1) * Wo])
                        t1 = t_pool.tile([128, Wo], BF16, tag=f"t1_{lvl}",
                                         name=f"t1_{lvl}")
                        wn = w // 3             # next-level width per unit
                        if lvl == 0:
                            av = a_pool.tile([128, Wo], BF16, tag=f"a{lvl}",
                                             name=f"a{lvl}")
                            dsts = [_sub(av[:], i3 * UPP * wn,
                                         [[wn, UPP], [1, wn]])
                                    for i3 in range(3)]
                            nxt = av[:]
                        else:
                            dsts = [_sub(R[2][:],
                                         i3 * T * UPP * wn + t * UPP * wn,
                                         [[wn, UPP], [1, wn]])
                                    for i3 in range(3)]
                            nxt = None
                        pc = lvl * 9
                        _avg_scatter(nc, dsts, t1, cur, phi_sb, pc + 0, w)
                        # detail branches on PE: 3 accumulating diag
                        # matmuls per 486-col chunk, j-outer so each
                        # weight loads once per (tile, branch); ACT casts
                        # psum fp32 -> SBUF bf16.
                        NCH = Wo // 486
                        for ri, dt_ in ((0, d1), (1, d2)):
                            # level 1 gets its own psum slots (ps6/ps7) so
                            # its matmuls don't wait on level-0 copy drains
                            s0 = 0 if lvl == 0 else 6
                            ps = [ps_pool.tile([128, 486], F32,
                                               tag=f"ps{s0 + c}",
                                               name=f"ps{s0 + c}")
                                  for c in range(NCH)]
                            for j in range(3):
                                bi = (lvl * 6 + ri * 3 + j) * 128
                                lhsT = dgt[:, bi: bi + 128]
                                for c in range(NCH):
                                    rhs = cur[:, j * Wo + c * 486:
                                              j * Wo + (c + 1) * 486]
                                    nc.tensor.matmul(
                                        ps[c][:], lhsT, rhs,
                                        start=(j == 0), stop=(j == 2))
                            for c in range(NCH):
                                dstv = dt_[:, c * 486:(c + 1) * 486]
                                if (c + ri) % 2 == 0:
                                    nc.scalar.copy(dstv, ps[c][:])
                                else:
                                    nc.vector.tensor_scalar_mul(
                                        dstv, ps[c][:], 1.0)
                        if lvl == 0:
                            for eng, dt_, off in ((nc.scalar, d1, OFF_D1[0]),
                                                  (nc.sync, d2, OFF_D2[0])):
                                dst = bass.AP(out, off + t * 128 * Wo,
                                              [[Wo, 128], [1, Wo]])
                                eng.dma_start(dst, dt_)
                        cur = nxt

                if copy_only:
                    return

                # staged detail DMAs for level 1 (one per branch)
                for lvl in (1,):
                    for b, off in ((1, OFF_D1[lvl]), (2, OFF_D2[lvl])):
                        st = ST[(lvl, b)]
                        n = st.shape[-1]
                        nc.sync.dma_start(
                            bass.AP(out, off, [[n, 128], [1, n]]), st[:])

                # ---- batched levels 2..6: run the whole avg chain
                # first so F7 (and thus the f7 gather + tail) starts as
                # early as possible; detail branches drain afterwards.
                for lvl in range(2, LK):
                    w = 3 ** (6 - lvl)
                    cur = R[lvl][:]
                    Wo = cur.shape[-1] // 3     # T*UPP*w
                    wn = w // 3
                    t1 = dd_pool.tile([128, Wo], BF16, tag=f"bt1_{lvl}",
                                      name=f"bt1_{lvl}")
                    pc = lvl * 9
                    if lvl < 6:
                        # avg into R[lvl+1], plane-major: the i3 blocks of
                        # the next resident are contiguous (t,u,j) runs.
                        dsts = [_sub(R[lvl + 1][:], i3 * T * UPP * wn,
                                     [[wn, T * UPP], [1, wn]])
                                for i3 in range(3)]
                        _avg_scatter(nc, dsts, t1, cur, phi_sb, pc + 0, w)
                    else:
                        # level 6: avg -> F7 [128, (t,u)] dense, no reorder
                        x0, x1, x2 = _thirds(cur)
                        c0, c1, c2 = _cols(phi_sb, pc + 0)
                        nc.vector.tensor_scalar_mul(t1[:, :Wo], x0, c0)
                        nc.vector.scalar_tensor_tensor(
                            t1[:, :Wo], x1, c1, t1[:, :Wo], MULT, ADD)
                        nc.vector.scalar_tensor_tensor(
                            F7[:], x2, c2, t1[:, :Wo], MULT, ADD)

                def batched_details():
                    for lvl in range(2, LK):
                        w = 3 ** (6 - lvl)
                        cur = R[lvl][:]
                        Wo = cur.shape[-1] // 3
                        d1 = ST[(lvl, 1)][:]
                        d2 = ST[(lvl, 2)][:]
                        t2 = dd_pool.tile([128, Wo], BF16, tag=f"bt2_{lvl}",
                                          name=f"bt2_{lvl}")
                        t3 = dd_pool.tile([128, Wo], BF16, tag=f"bt3_{lvl}",
                                          name=f"bt3_{lvl}")
                        t4 = dd_pool.tile([128, Wo], BF16, tag=f"bt4_{lvl}",
                                          name=f"bt4_{lvl}")
                        t5 = dd_pool.tile([128, Wo], BF16, tag=f"bt5_{lvl}",
                                          name=f"bt5_{lvl}")
                        pc = lvl * 9
                        _det_branch(nc, d1, t2, t3, cur, phi_sb, pc + 3)
                        _det_branch(nc, d2, t4, t5, cur, phi_sb, pc + 6)
                        for b, off in ((1, OFF_D1[lvl]), (2, OFF_D2[lvl])):
                            st = ST[(lvl, b)]
                            n = st.shape[-1]
                            nc.scalar.dma_start(
                                bass.AP(out, off, [[n, 128], [1, n]]),
                                st[:])

                # f7 slab out: value for unit g = t*256 + p*2 + u
                dstf = bass.AP(out, OFF_F7,
                               [[UPP, 128], [128 * UPP, T], [1, UPP]])
                nc.sync.dma_start(dstf, F7[:].rearrange("p (t j) -> p t j",
                                                        t=T))

                if not merge_tail:
                    batched_details()
                if merge_tail:
                    f7_loc, f7_all, f7_flat = dram_tiles()
                    nc.sync.dma_start(
                        bass.AP(f7_loc.tensor, f7_loc[:].offset,
                                [[UPP, 128], [128 * UPP, T], [1, UPP]]),
                        F7[:].rearrange("p (t j) -> p t j", t=T))
                    if not hoist_collective:
                        collective(f7_loc, f7_all)
                    # one strided DMA compacts padded slabs (U0[j]=j*2461)
                    nc.sync.dma_start(
                        f7_flat[:],
                        bass.AP(f7_all.tensor, f7_all[:].offset,
                                [[PAD_UNITS, NCORES], [1, 2461]]))
                    _emit_tail(nc, tail_pool, phi_sb, f7_flat.tensor,
                               f7_flat[:].offset, tail_out)
                    batched_details()

            if nrep == 1:
                body()
            else:
                if merge_tail and hoist_collective:
                    f7_loc, f7_all, _ = dram_tiles()
                    collective(f7_loc, f7_all)
                with tc.For_i(0, nrep, 1):
                    body()

    return nc


def _phi_from_inputs(Phi_P: np.ndarray) -> np.ndarray:
    Q = np.stack([np.linalg.qr(Phi_P[i].T.astype(np.float32))[0]
                  for i in range(Phi_P.shape[0])])
    return np.transpose(Q, (0, 2, 1)).astype(np.float32)


# ternary digit-reversal permutations: REV[k][i] = reverse of i in k digits
_REV = {}
for _k in range(1, 8):
    idx = np.arange(3 ** _k)
    r = np.zeros_like(idx)
    ii = idx.copy()
    for _d in range(_k):
        r = r * 3 + (ii % 3)
        ii //= 3
    _REV[_k] = r

_CACHE = {}


def _make_dg(Phi):
    """Diag weight blocks for the PE detail branches of levels 0..1."""
    dg = np.zeros((12, 128, 128), np.float32)
    for lvl in range(2):
        for ri, r in enumerate((1, 2)):
            for j in range(3):
                np.fill_diagonal(dg[lvl * 6 + ri * 3 + j], Phi[lvl, r, j])
    return dg.transpose(1, 0, 2).reshape(128, 12 * 128).astype(NPBF)


def kernel(f: np.ndarray, Phi_P: np.ndarray) -> np.ndarray:
    f = np.asarray(f, dtype=np.float32).ravel()
    Phi = _phi_from_inputs(np.asarray(Phi_P, dtype=np.float32))

    phi_all = np.broadcast_to(
        Phi.reshape(1, NL * 9), (128, NL * 9)).copy()
    dg = _make_dg(Phi)

    if "main" not in _CACHE:
        _CACHE["main"] = build_main()
        _split_multi_waits(_CACHE["main"])
    nc_main = _CACHE["main"]

    rev7 = _REV[7]
    in_maps = []
    for k in range(NCORES):
        lo = CORE_U0[k] * UNIT
        n = CORE_UN[k] * UNIT
        xk = np.zeros(PAD_UNITS * UNIT, dtype=np.float32)
        xk[:n] = f[lo:lo + n]
        # per unit: digit-reverse; per pair: plane-major [3][u][729]
        u = xk.reshape(PAD_UNITS // UPP, UPP, UNIT)[:, :, rev7]
        u = u.reshape(PAD_UNITS // UPP, UPP, 3, UNIT // 3)
        u = np.ascontiguousarray(u.transpose(0, 2, 1, 3))
        in_maps.append({"x": u.reshape(-1).astype(NPBF), "phi": phi_all,
                        "dg": dg})

    res = run_bass_kernel_spmd(nc_main, in_maps, list(range(NCORES)))

    f_hat = np.empty(3 ** NL, dtype=np.float32)
    for k in range(NCORES):
        ok = res.results[k]["out"]
        u0, un = CORE_U0[k], CORE_UN[k]
        for i in range(LK):
            w = 3 ** (6 - i)
            base = 3 ** (15 - i)
            for off, dst0 in ((OFF_D1[i], base), (OFF_D2[i], 2 * base)):
                blk = np.asarray(ok[off: off + PAD_UNITS * w])
                if i == 0:
                    # level 0 DMAs per tile: [t][p][u][w'] already g-major
                    blk = blk.reshape(PAD_UNITS, w)
                else:
                    # staged: [p][t][u][w'] -> [t][p][u][w'] = g-major
                    blk = blk.reshape(128, T, UPP, w).transpose(1, 0, 2, 3)
                    blk = blk.reshape(PAD_UNITS, w)
                if w > 1:
                    blk = blk[:, _REV[6 - i]]
                f_hat[dst0 + u0 * w: dst0 + (u0 + un) * w] = \
                    blk[:un].reshape(-1).astype(np.float32)
    f_hat[:NUNITS] = np.asarray(
        res.results[0]["tail"]).astype(np.float32)
    # device tail stops at level 12; finish levels 12..15 on host
    cur = f_hat[0:81].copy()
    for L in range(12, NL):
        fm = cur.reshape(-1, 3)
        base = 3 ** (15 - L)
        f_hat[base: base + fm.shape[0]] = fm @ Phi[L, 1, :]
        f_hat[2 * base: 2 * base + fm.shape[0]] = fm @ Phi[L, 2, :]
        cur = fm @ Phi[L, 0, :]
    f_hat[0] = cur[0]
    return f_hat
